# revision 2
# baseline (speedup 1.0000x reference)
"""Grouped (block-diagonal) linear kernel for Trainium2, 8 NeuronCores.

Problem: x [4, 4096, 4096] f32, weight [128, 32, 32], bias [128, 32].
out[b,s,n,o] = sum_i x[b,s,n*32+i] * weight[n,i,o] + bias[n,o], flattened back
to [4, 4096, 4096].

Sharding: the 128 blocks are split across 8 cores (16 blocks = 512 features
per core). Each core reads its own 512-column slice of x and writes the
matching 512-column slice of the output; results are concatenated on host.

Per-core kernel (memory-bound design, ~32 MB in + 32 MB out per core):
  - DMA 1024-token chunks [128p x 4096f] (2 MB per transfer, natural layout).
  - For each 128-token sub-chunk: PE transposes the 4 [128,128] feature
    groups (features -> partitions), ACT copies PSUM->SBUF, then 4 matmuls
    lhsT=xT (stationary) x rhs=block-diag(weights) accumulate into PSUM in
    natural [token, feature] layout. DVE adds bias during the PSUM->SBUF copy.
  - DMA the chunk back out.
"""

import numpy as np

import concourse.bass as bass
import concourse.bacc as bacc
import concourse.mybir as mybir
import concourse.tile as tile

B, S = 4, 4096
IN_F = OUT_F = 4096
NB, IPB, OPB = 128, 32, 32
NCORES = 8
BPC = NB // NCORES            # blocks per core = 16
FPC = BPC * IPB               # features per core = 512
TOK = B * S                   # tokens = 16384
GROUPS = FPC // 128           # 128-feature groups per core = 4
BLOCKS_PER_GROUP = 128 // IPB  # 4

F32 = mybir.dt.float32
F16 = mybir.dt.float16


def build_nc_v2(
    loop_reps: int = 1,
    ch: int = 2048,            # tokens per chunk
    dve_cols: int = 288,       # PSUM->SBUF columns copied by DVE (rest on ACT)
    bias_pe: bool = True,      # add bias via K=1 ones-matmul on PE
    psum_bufs: int = 5,
    mode: str = "dmat",        # "dmat" (DMA-transpose in) | "pet" (PE transpose)
    xbufs: int = 2,
    obufs: int = 2,
):
    """fp16 redesign, block-sharded (16 blocks = 512 features per core).

    mode="dmat": host packs x fp16 as [nchunk, 4*ch, 128] (group-major,
    token-permuted); one 16-bit xbar DMA-transpose per chunk lands features
    on partitions directly -- no PE transpose, no xT PSUM round trip.
    Per 128-token sub-chunk: 1 bias matmul (K=1 ones x bias row, start=True)
    + 4 fp16 block-diag matmuls (1 cyc/row) accumulate into a PSUM bank in
    [token, feature] layout; DVE/ACT split the PSUM->SBUF fp16 downconvert
    copy. Input and output DMAs are fully contiguous.

    mode="pet": natural-layout fp16 input; PE transposes (1 cyc/row) feed
    the same matmul pipeline; ACT does the xT PSUM->SBUF copy, DVE the
    output copy.
    """
    assert TOK % ch == 0 and ch % 128 == 0
    nchunk = TOK // ch
    sub = ch // 128
    if not bias_pe:
        dve_cols = FPC

    nc = bacc.Bacc(
        "TRN2", target_bir_lowering=False, debug=False, num_devices=NCORES
    )
    if mode == "dmat":
        xg = nc.dram_tensor("xg", [nchunk, GROUPS * ch, 128], F16,
                            kind="ExternalInput").ap()
    else:
        xs = nc.dram_tensor("xs", [TOK, FPC], F16, kind="ExternalInput").ap()
        xs3 = xs.rearrange("(c p a) f -> c p (a f)", p=128, a=sub)
        idn = nc.dram_tensor("idn", [128, 128], F16, kind="ExternalInput").ap()
    wt_d = nc.dram_tensor("wt", [128, FPC], F16, kind="ExternalInput").ap()
    b1_d = nc.dram_tensor("b1", [1, FPC], F16, kind="ExternalInput").ap()
    on_d = nc.dram_tensor("on", [1, 128], F16, kind="ExternalInput").ap()
    bt_d = nc.dram_tensor("bt", [128, FPC], F32, kind="ExternalInput").ap()
    out = nc.dram_tensor("out", [TOK, FPC], F16, kind="ExternalOutput").ap()
    out3 = out.rearrange("(c p a) f -> c p (a f)", p=128, a=sub)

    import contextlib

    with tile.TileContext(nc) as tc:
        with (
            tc.tile_pool(name="const", bufs=1) as cpool,
            tc.tile_pool(name="xin", bufs=xbufs) as xpool,
            tc.tile_pool(name="oout", bufs=obufs) as opool,
            tc.tile_pool(name="xt", bufs=3) as xtpool,
            tc.tile_pool(name="ps", bufs=psum_bufs, space="PSUM") as pspool,
            tc.tile_pool(name="psx", bufs=3, space="PSUM") as psxpool,
        ):
            wt = cpool.tile([128, FPC], F16)
            nc.sync.dma_start(out=wt[:], in_=wt_d)
            b1t = cpool.tile([1, FPC], F16)
            nc.sync.dma_start(out=b1t[:], in_=b1_d)
            ot1 = cpool.tile([1, 128], F16)
            nc.sync.dma_start(out=ot1[:], in_=on_d)
            if not bias_pe:
                bt = cpool.tile([128, FPC], F32)
                nc.sync.dma_start(out=bt[:], in_=bt_d)
            if mode == "pet":
                it = cpool.tile([128, 128], F16)
                nc.sync.dma_start(out=it[:], in_=idn)

            loop_ctx = (
                tc.For_i(
                    0, loop_reps, 1,
                    hint_engines=(mybir.EngineType.PE, mybir.EngineType.DVE),
                )
                if loop_reps > 1
                else contextlib.nullcontext()
            )
            with loop_ctx:
                for c in range(nchunk):
                    if mode == "dmat":
                        xt = xpool.tile([128, GROUPS * ch], F16)
                        nc.sync.dma_start(out=xt[:], in_=xg[c], transpose=True)
                    else:
                        x_in = xpool.tile([128, sub * FPC], F16)
                        nc.sync.dma_start(out=x_in[:], in_=xs3[c])
                    ot = opool.tile([128, sub * FPC], F16)
                    for s in range(sub):
                        if mode == "pet":
                            xt_ps = psxpool.tile([128, FPC], F16)
                            for g in range(GROUPS):
                                nc.tensor.transpose(
                                    xt_ps[:, bass.ts(g, 128)],
                                    x_in[:, s * FPC + g * 128:
                                         s * FPC + (g + 1) * 128],
                                    it[:],
                                )
                            xt_sb = xtpool.tile([128, FPC], F16)
                            nc.scalar.copy(xt_sb[:], xt_ps[:])

                        def lhs(g):
                            if mode == "dmat":
                                return xt[:, g * ch + s * 128:
                                          g * ch + (s + 1) * 128]
                            return xt_sb[:, bass.ts(g, 128)]

                        ps = pspool.tile([128, FPC], F32)
                        if bias_pe:
                            nc.tensor.matmul(
                                ps[:], lhsT=ot1[:], rhs=b1t[:],
                                start=True, stop=False,
                            )
                        for g in range(GROUPS):
                            nc.tensor.matmul(
                                ps[:, bass.ts(g, 128)],
                                lhsT=lhs(g),
                                rhs=wt[:, bass.ts(g, 128)],
                                start=not bias_pe,
                                stop=True,
                            )
                        if bias_pe:
                            if dve_cols:
                                nc.vector.tensor_copy(
                                    ot[:, s * FPC: s * FPC + dve_cols],
                                    ps[:, 0:dve_cols],
                                )
                            if dve_cols < FPC:
                                nc.scalar.copy(
                                    ot[:, s * FPC + dve_cols: (s + 1) * FPC],
                                    ps[:, dve_cols:FPC],
                                )
                        else:
                            nc.vector.tensor_add(
                                ot[:, bass.ts(s, FPC)], ps[:], bt[:]
                            )
                    nc.scalar.dma_start(out=out3[c], in_=ot[:])
    nc.compile()
    return nc


def prep_in_maps_v2(x, weight, bias, ch: int = 2048, mode: str = "dmat"):
    """Per-core fp16 input maps for build_nc_v2."""
    x2 = np.asarray(x, np.float32).reshape(TOK, IN_F)
    w = np.asarray(weight, np.float32)
    b = np.asarray(bias, np.float32)
    nchunk = TOK // ch
    sub = ch // 128
    ones = np.ones((1, 128), np.float16)
    ident = np.eye(128, dtype=np.float16)
    maps = []
    for m in range(NCORES):
        xs = x2[:, m * FPC:(m + 1) * FPC].astype(np.float16)   # [TOK, 512]
        wm = w[m * BPC:(m + 1) * BPC]                          # [16, 32, 32]
        wg = np.zeros((128, FPC), np.float16)
        for g in range(GROUPS):
            for a in range(BLOCKS_PER_GROUP):
                wg[a * 32:(a + 1) * 32,
                   g * 128 + a * 32: g * 128 + (a + 1) * 32] = wm[4 * g + a]
        b1 = b[m * BPC:(m + 1) * BPC].reshape(1, FPC).astype(np.float16)
        btm = np.ascontiguousarray(
            np.broadcast_to(b[m * BPC:(m + 1) * BPC].reshape(FPC), (128, FPC))
        ).astype(np.float32)
        mp = {"wt": wg, "b1": b1, "on": ones, "bt": btm}
        if mode == "dmat":
            # row (c, g, s*128+p) of xg = features [g*128:(g+1)*128] of
            # original token c*ch + p*sub + s
            xgm = xs.reshape(nchunk, 128, sub, GROUPS, 128)    # [c,p,s,g,u]
            xgm = np.ascontiguousarray(xgm.transpose(0, 3, 2, 1, 4))
            mp["xg"] = xgm.reshape(nchunk, GROUPS * ch, 128)
        else:
            mp["xs"] = xs
            mp["idn"] = ident
        maps.append(mp)
    return maps


def kernel_v2(inputs, ch: int = 2048, mode: str = "dmat", **bkw) -> np.ndarray:
    from concourse.bass_utils import run_bass_kernel_spmd

    nc = build_nc_v2(ch=ch, mode=mode, **bkw)
    in_maps = prep_in_maps_v2(inputs["x"], inputs["weight"], inputs["bias"],
                              ch=ch, mode=mode)
    res = run_bass_kernel_spmd(nc, in_maps, core_ids=list(range(NCORES)))
    outs = [res.results[m]["out"].astype(np.float32) for m in range(NCORES)]
    full = np.concatenate(outs, axis=1)           # [16384, 4096]
    return full.reshape(B, S, OUT_F)


def build_nc(
    tok: int = TOK,
    chunk_tok: int = 1024,
    reps: int = 1,
    loop_reps: int = 1,
    use_f32r: bool = False,
    variant: str = "full",      # full | dma | nomm | notr  (bisection variants)
    copy_engine: str = "vector",  # engine for the xT PSUM->SBUF copy
):
    """Build the per-core Bass program (SPMD: same program, per-core data).

    reps: python-unrolled repetitions of the whole pass (for timing).
    loop_reps: hardware For_i loop repetitions of the whole pass (for timing
    with constant instruction count).
    use_f32r: stream operands as float32r (same bits as fp32, faster PE
    streaming mode) and run the matmuls as zero-padded pairs with a 256-wide
    moving dim, where f32r hits 1 cycle/row instead of fp32's 4.
    """
    assert tok % chunk_tok == 0 and chunk_tok % 128 == 0
    nchunk = tok // chunk_tok
    sub = chunk_tok // 128     # 128-token sub-chunks per chunk
    XD = mybir.dt.float32r if use_f32r else F32

    nc = bacc.Bacc(
        "TRN2", target_bir_lowering=False, debug=False, num_devices=NCORES
    )
    xs = nc.dram_tensor("xs", [tok, FPC], XD, kind="ExternalInput").ap()
    if use_f32r:
        wpad = nc.dram_tensor(
            "wpad", [GROUPS, 128, 256], XD, kind="ExternalInput"
        ).ap()
    else:
        wbd = nc.dram_tensor("wbd", [GROUPS, 128, 128], F32, kind="ExternalInput").ap()
    bb = nc.dram_tensor("bb", [128, FPC], F32, kind="ExternalInput").ap()
    idn = nc.dram_tensor("idn", [128, 128], XD, kind="ExternalInput").ap()
    out = nc.dram_tensor("out", [tok, FPC], F32, kind="ExternalOutput").ap()

    xs3 = xs.rearrange("(c a p) f -> c p a f", a=sub, p=128)
    out3 = out.rearrange("(c a p) f -> c p a f", a=sub, p=128)

    with tile.TileContext(nc) as tc:
        with (
            tc.tile_pool(name="const", bufs=1) as cpool,
            tc.tile_pool(name="xin", bufs=2) as xpool,
            tc.tile_pool(name="oout", bufs=2) as opool,
            tc.tile_pool(name="xt", bufs=3) as xtpool,
            tc.tile_pool(name="ps", bufs=2, space="PSUM") as pspool,
        ):
            if use_f32r:
                wt = cpool.tile([128, GROUPS * 256], XD)
                nc.sync.dma_start(
                    out=wt[:].rearrange("p (g m) -> p g m", g=GROUPS),
                    in_=wpad.rearrange("g k m -> k g m"),
                )
            else:
                wt = cpool.tile([128, GROUPS * 128], F32)
                nc.sync.dma_start(
                    out=wt[:].rearrange("p (g m) -> p g m", g=GROUPS),
                    in_=wbd.rearrange("g k m -> k g m"),
                )
            bt = cpool.tile([128, FPC], F32)
            nc.sync.dma_start(out=bt[:], in_=bb)
            it = cpool.tile([128, 128], XD)
            nc.sync.dma_start(out=it[:], in_=idn)

            import contextlib

            loop_ctx = (
                tc.For_i(
                    0,
                    loop_reps,
                    1,
                    hint_engines=(mybir.EngineType.PE, mybir.EngineType.Activation),
                )
                if loop_reps > 1
                else contextlib.nullcontext()
            )
            with loop_ctx:
                for _ in range(reps):
                    for c in range(nchunk):
                        x_in = xpool.tile([128, sub * FPC], XD)
                        nc.sync.dma_start(
                            out=x_in[:].rearrange("p (a f) -> p a f", a=sub),
                            in_=xs3[c],
                        )
                        if variant == "dma":
                            nc.scalar.dma_start(
                                out=out3[c],
                                in_=x_in[:].rearrange("p (a f) -> p a f", a=sub),
                            )
                            continue
                        cp_fn = (
                            nc.scalar.copy
                            if copy_engine == "scalar"
                            else nc.vector.tensor_copy
                        )
                        ot = opool.tile([128, sub * FPC], F32)
                        for s in range(sub):
                            if variant != "notr":
                                xT_ps = pspool.tile([128, FPC], XD)
                                for g in range(GROUPS):
                                    nc.tensor.transpose(
                                        xT_ps[:, bass.ts(g, 128)],
                                        x_in[
                                            :,
                                            s * FPC + g * 128 : s * FPC + (g + 1) * 128,
                                        ],
                                        it[:],
                                    )
                                xT_sb = xtpool.tile([128, FPC], XD)
                                cp_fn(xT_sb[:], xT_ps[:])
                            else:
                                xT_sb = x_in[:, bass.ts(s, FPC)]
                            if variant == "nomm":
                                nc.vector.tensor_add(
                                    ot[:, bass.ts(s, FPC)], xT_ps[:], bt[:]
                                )
                                continue
                            o_ps = pspool.tile([128, FPC], F32)
                            if use_f32r:
                                for p in range(GROUPS // 2):
                                    for h in range(2):
                                        nc.tensor.matmul(
                                            o_ps[:, bass.ts(p, 256)],
                                            lhsT=xT_sb[:, bass.ts(2 * p + h, 128)],
                                            rhs=wt[:, bass.ts(2 * p + h, 256)],
                                            start=(h == 0),
                                            stop=(h == 1),
                                        )
                            else:
                                for g in range(GROUPS):
                                    nc.tensor.matmul(
                                        o_ps[:, bass.ts(g, 128)],
                                        lhsT=xT_sb[:, bass.ts(g, 128)],
                                        rhs=wt[:, bass.ts(g, 128)],
                                        start=True,
                                        stop=True,
                                    )
                            nc.vector.tensor_add(
                                ot[:, bass.ts(s, FPC)], o_ps[:], bt[:]
                            )
                        nc.scalar.dma_start(
                            out=out3[c],
                            in_=ot[:].rearrange("p (a f) -> p a f", a=sub),
                        )
    nc.compile()
    return nc


def build_nc_tok(
    tpc: int = TOK // NCORES,
    loop_reps: int = 1,
    use_f32r: bool = False,
    qf: int = 1024,             # features per PSUM quarter (multiple of 256)
    psum_bufs: int = 2,
    variant: str = "full",      # full | dma
    mm_transpose_mode: bool = False,  # run matmuls with is_transpose=True
):
    """Token-sharded per-core program: each core owns tpc tokens x all 4096
    features. DMA is fully contiguous (16 KB per partition per transfer)."""
    assert tpc % 128 == 0
    nsub = tpc // 128
    ngrp = IN_F // 128          # 32 groups of 128 features
    nq = IN_F // qf             # PSUM quarters per sub-chunk
    gq = qf // 128              # groups per quarter
    XD = mybir.dt.float32r if use_f32r else F32

    nc = bacc.Bacc(
        "TRN2", target_bir_lowering=False, debug=False, num_devices=NCORES
    )
    xs = nc.dram_tensor("xs", [tpc, IN_F], XD, kind="ExternalInput").ap()
    if use_f32r:
        wpad = nc.dram_tensor(
            "wpad", [ngrp, 128, 256], XD, kind="ExternalInput"
        ).ap()
    else:
        wbd = nc.dram_tensor("wbd", [ngrp, 128, 128], F32, kind="ExternalInput").ap()
    bb = nc.dram_tensor("bb", [128, IN_F], F32, kind="ExternalInput").ap()
    idn = nc.dram_tensor("idn", [128, 128], XD, kind="ExternalInput").ap()
    out = nc.dram_tensor("out", [tpc, IN_F], F32, kind="ExternalOutput").ap()

    xs2 = xs.rearrange("(c p) f -> c p f", p=128)
    out2 = out.rearrange("(c p) f -> c p f", p=128)

    with tile.TileContext(nc) as tc:
        with (
            tc.tile_pool(name="const", bufs=1) as cpool,
            tc.tile_pool(name="xin", bufs=3) as xpool,
            tc.tile_pool(name="oout", bufs=3) as opool,
            tc.tile_pool(name="xt", bufs=3) as xtpool,
            tc.tile_pool(name="ps", bufs=psum_bufs, space="PSUM") as pspool,
        ):
            if use_f32r:
                wt = cpool.tile([128, ngrp * 256], XD)
                nc.sync.dma_start(
                    out=wt[:].rearrange("p (g m) -> p g m", g=ngrp),
                    in_=wpad.rearrange("g k m -> k g m"),
                )
            else:
                wt = cpool.tile([128, ngrp * 128], F32)
                nc.sync.dma_start(
                    out=wt[:].rearrange("p (g m) -> p g m", g=ngrp),
                    in_=wbd.rearrange("g k m -> k g m"),
                )
            bt = cpool.tile([128, IN_F], F32)
            nc.sync.dma_start(out=bt[:], in_=bb)
            it = cpool.tile([128, 128], XD)
            nc.sync.dma_start(out=it[:], in_=idn)

            import contextlib

            loop_ctx = (
                tc.For_i(
                    0,
                    loop_reps,
                    1,
                    hint_engines=(mybir.EngineType.PE, mybir.EngineType.DVE),
                )
                if loop_reps > 1
                else contextlib.nullcontext()
            )
            with loop_ctx:
                for c in range(nsub):
                    x_in = xpool.tile([128, IN_F], XD)
                    nc.sync.dma_start(out=x_in[:], in_=xs2[c])
                    if variant == "dma":
                        nc.scalar.dma_start(out=out2[c], in_=x_in[:])
                        continue
                    ot = opool.tile([128, IN_F], F32)
                    for q in range(nq):
                        xT_ps = pspool.tile([128, qf], XD)
                        for g in range(gq):
                            nc.tensor.transpose(
                                xT_ps[:, bass.ts(g, 128)],
                                x_in[:, q * qf + g * 128 : q * qf + (g + 1) * 128],
                                it[:],
                            )
                        xT_sb = xtpool.tile([128, qf], XD)
                        nc.vector.tensor_copy(xT_sb[:], xT_ps[:])
                        o_ps = pspool.tile([128, qf], F32)
                        if use_f32r:
                            for p in range(gq // 2):
                                for h in range(2):
                                    nc.tensor.matmul(
                                        o_ps[:, bass.ts(p, 256)],
                                        lhsT=xT_sb[:, bass.ts(2 * p + h, 128)],
                                        rhs=wt[
                                            :,
                                            (q * gq + 2 * p + h)
                                            * 256 : (q * gq + 2 * p + h + 1)
                                            * 256,
                                        ],
                                        start=(h == 0),
                                        stop=(h == 1),
                                    )
                        else:
                            for g in range(gq):
                                nc.tensor.matmul(
                                    o_ps[:, bass.ts(g, 128)],
                                    lhsT=xT_sb[:, bass.ts(g, 128)],
                                    rhs=wt[:, bass.ts(q * gq + g, 128)],
                                    start=True,
                                    stop=True,
                                    is_transpose=mm_transpose_mode or None,
                                )
                        nc.vector.tensor_add(
                            ot[:, bass.ts(q, qf)], o_ps[:], bt[:, bass.ts(q, qf)]
                        )
                    nc.scalar.dma_start(out=out2[c], in_=ot[:])
    nc.compile()
    return nc


def build_nc_ht(
    tpc: int = TOK // NCORES,
    loop_reps: int = 1,
    win_tok: int = 256,         # tokens per input window (one 4MB DMA each)
    psum_bufs: int = 6,
    use_f32r: bool = False,
):
    """Host-transposed per-core program: x arrives feature-major [4096, tpc],
    so features land on partitions straight from DMA — no on-chip transpose,
    no PSUM round-trip for inputs. Token-sharded across cores."""
    assert tpc % win_tok == 0 and win_tok % 128 == 0
    nwin = tpc // win_tok
    tc_per_win = win_tok // 128
    ngrp = IN_F // 128          # 32
    XD = mybir.dt.float32r if use_f32r else F32

    nc = bacc.Bacc(
        "TRN2", target_bir_lowering=False, debug=False, num_devices=NCORES
    )
    xt = nc.dram_tensor("xt", [IN_F, tpc], XD, kind="ExternalInput").ap()
    if use_f32r:
        wpad = nc.dram_tensor(
            "wpad", [ngrp, 128, 256], XD, kind="ExternalInput"
        ).ap()
    else:
        wbd = nc.dram_tensor("wbd", [ngrp, 128, 128], F32, kind="ExternalInput").ap()
    bb = nc.dram_tensor("bb", [128, IN_F], F32, kind="ExternalInput").ap()
    out = nc.dram_tensor("out", [tpc, IN_F], F32, kind="ExternalOutput").ap()

    xt4 = xt.rearrange("(g p) t -> p g t", g=ngrp, p=128)  # [128, 32, tpc]
    out2 = out.rearrange("(c p) f -> c p f", p=128)

    with tile.TileContext(nc) as tc:
        with (
            tc.tile_pool(name="const", bufs=1) as cpool,
            tc.tile_pool(name="xin", bufs=2) as xpool,
            tc.tile_pool(name="oout", bufs=2) as opool,
            tc.tile_pool(name="ps", bufs=psum_bufs, space="PSUM") as pspool,
        ):
            if use_f32r:
                wt = cpool.tile([128, ngrp * 256], XD)
                nc.sync.dma_start(
                    out=wt[:].rearrange("p (g m) -> p g m", g=ngrp),
                    in_=wpad.rearrange("g k m -> k g m"),
                )
            else:
                wt = cpool.tile([128, ngrp * 128], F32)
                nc.sync.dma_start(
                    out=wt[:].rearrange("p (g m) -> p g m", g=ngrp),
                    in_=wbd.rearrange("g k m -> k g m"),
                )
            bt = cpool.tile([128, IN_F], F32)
            nc.sync.dma_start(out=bt[:], in_=bb)

            import contextlib

            loop_ctx = (
                tc.For_i(
                    0,
                    loop_reps,
                    1,
                    hint_engines=(mybir.EngineType.PE, mybir.EngineType.DVE),
                )
                if loop_reps > 1
                else contextlib.nullcontext()
            )
            with loop_ctx:
                for w in range(nwin):
                    xw = xpool.tile([128, ngrp * win_tok], XD)
                    nc.sync.dma_start(
                        out=xw[:].rearrange("p (g t) -> p g t", g=ngrp),
                        in_=xt4[:, :, w * win_tok : (w + 1) * win_tok],
                    )
                    for tci in range(tc_per_win):
                        ot = opool.tile([128, IN_F], F32)
                        for q in range(IN_F // 512):
                            o_ps = pspool.tile([128, 512], F32)
                            if use_f32r:
                                for p in range(2):
                                    for h in range(2):
                                        g = q * 4 + 2 * p + h
                                        nc.tensor.matmul(
                                            o_ps[:, bass.ts(p, 256)],
                                            lhsT=xw[
                                                :,
                                                g * win_tok
                                                + tci * 128 : g * win_tok
                                                + tci * 128
                                                + 128,
                                            ],
                                            rhs=wt[:, bass.ts(g, 256)],
                                            start=(h == 0),
                                            stop=(h == 1),
                                        )
                            else:
                                for j in range(4):
                                    g = q * 4 + j
                                    nc.tensor.matmul(
                                        o_ps[:, bass.ts(j, 128)],
                                        lhsT=xw[
                                            :,
                                            g * win_tok
                                            + tci * 128 : g * win_tok
                                            + tci * 128
                                            + 128,
                                        ],
                                        rhs=wt[:, bass.ts(g, 128)],
                                        start=True,
                                        stop=True,
                                    )
                            nc.vector.tensor_add(
                                ot[:, bass.ts(q, 512)],
                                o_ps[:],
                                bt[:, bass.ts(q, 512)],
                            )
                        nc.scalar.dma_start(
                            out=out2[w * tc_per_win + tci], in_=ot[:]
                        )
    nc.compile()
    return nc


def prep_in_maps_ht(x, weight, bias, use_f32r: bool = False):
    """Host-transposed inputs: per-core feature-major x slice."""
    x = np.asarray(x, dtype=np.float32).reshape(-1, IN_F)
    weight = np.asarray(weight, dtype=np.float32)
    bias = np.asarray(bias, dtype=np.float32)
    tpc = x.shape[0] // NCORES

    ngrp = IN_F // 128
    bpg = 128 // IPB
    wg = np.zeros((ngrp, 128, 128), np.float32)
    for g in range(ngrp):
        for a in range(bpg):
            wg[g, 32 * a : 32 * a + 32, 32 * a : 32 * a + 32] = weight[bpg * g + a]
    bbm = np.ascontiguousarray(np.broadcast_to(bias.reshape(IN_F), (128, IN_F)))
    maps = []
    for m in range(NCORES):
        xtm = np.ascontiguousarray(x[m * tpc : (m + 1) * tpc].T)
        mp = {"xt": xtm, "bb": bbm}
        if use_f32r:
            wp = np.zeros((ngrp, 128, 256), np.float32)
            for qq in range(ngrp):
                h = qq % 2
                wp[qq, :, 128 * h : 128 * h + 128] = wg[qq]
            mp["wpad"] = wp
        else:
            mp["wbd"] = wg
        maps.append(mp)
    return maps


def prep_in_maps_tok(x, weight, bias):
    """Token-sharded inputs: per-core contiguous token slice, shared weights."""
    x = np.ascontiguousarray(np.asarray(x, dtype=np.float32).reshape(-1, IN_F))
    weight = np.asarray(weight, dtype=np.float32)
    bias = np.asarray(bias, dtype=np.float32)
    ident = np.eye(128, dtype=np.float32)
    tpc = x.shape[0] // NCORES

    ngrp = IN_F // 128
    bpg = 128 // IPB            # blocks per 128-feature group = 4
    wg = np.zeros((ngrp, 128, 128), np.float32)
    for g in range(ngrp):
        for a in range(bpg):
            wg[g, 32 * a : 32 * a + 32, 32 * a : 32 * a + 32] = weight[bpg * g + a]
    wp = np.zeros((ngrp, 128, 256), np.float32)
    for qq in range(ngrp):
        h = qq % 2
        wp[qq, :, 128 * h : 128 * h + 128] = wg[qq]
    bbm = np.ascontiguousarray(
        np.broadcast_to(bias.reshape(IN_F), (128, IN_F))
    )
    return [
        {
            "xs": x[m * tpc : (m + 1) * tpc],
            "wbd": wg,
            "wpad": wp,
            "bb": bbm,
            "idn": ident,
        }
        for m in range(NCORES)
    ]


def prep_in_maps(x, weight, bias, tok: int = TOK):
    """Split full inputs into 8 per-core input maps (host-side numpy)."""
    x = np.asarray(x, dtype=np.float32).reshape(-1, IN_F)[:tok]
    weight = np.asarray(weight, dtype=np.float32)
    bias = np.asarray(bias, dtype=np.float32)
    ident = np.eye(128, dtype=np.float32)

    in_maps = []
    for m in range(NCORES):
        xs = np.ascontiguousarray(x[:, m * FPC : (m + 1) * FPC])
        wm = weight[m * BPC : (m + 1) * BPC]          # [16, 32, 32]
        wg = np.zeros((GROUPS, 128, 128), np.float32)
        for g in range(GROUPS):
            for a in range(BLOCKS_PER_GROUP):
                wg[g, 32 * a : 32 * a + 32, 32 * a : 32 * a + 32] = wm[
                    BLOCKS_PER_GROUP * g + a
                ]
        # zero-padded pairs for the f32r N=256 matmul path: entry q = 2p+h
        # holds group (2p+h)'s weights in column half h, zeros in the other.
        wp = np.zeros((GROUPS, 128, 256), np.float32)
        for q in range(GROUPS):
            h = q % 2
            wp[q, :, 128 * h : 128 * h + 128] = wg[q]
        bm = bias[m * BPC : (m + 1) * BPC].reshape(FPC)
        bbm = np.ascontiguousarray(np.broadcast_to(bm, (128, FPC)))
        in_maps.append({"xs": xs, "wbd": wg, "wpad": wp, "bb": bbm, "idn": ident})
    return in_maps


def kernel(**inputs) -> np.ndarray:
    from concourse.bass_utils import run_bass_kernel_spmd

    nc = build_nc()
    in_maps = prep_in_maps(inputs["x"], inputs["weight"], inputs["bias"])
    res = run_bass_kernel_spmd(nc, in_maps, core_ids=list(range(NCORES)))
    outs = [res.results[m]["out"] for m in range(NCORES)]
    full = np.concatenate(outs, axis=1)           # [16384, 4096]
    return full.reshape(B, S, OUT_F)



# revision 11
# speedup vs baseline: 1.2540x; 1.2540x over previous
"""Grouped (block-diagonal) linear kernel for Trainium2, 8 NeuronCores.

Problem: x [4, 4096, 4096] f32, weight [128, 32, 32], bias [128, 32].
out[b,s,n,o] = sum_i x[b,s,n*32+i] * weight[n,i,o] + bias[n,o], flattened back
to [4, 4096, 4096].

Sharding: the 128 blocks are split across 8 cores (16 blocks = 512 features
per core). Each core reads its own 512-column slice of x and writes the
matching 512-column slice of the output; results are concatenated on host.

Current design (build_nc_v2, mode="pet"): fp16 compute, 16 MB in + 16 MB out
per core (host converts x to fp16 and upconverts the fp16 result; matmul
accumulation stays fp32 in PSUM, rel err ~6e-4 vs the 2e-2 gate):
  - x fp16 in natural [token, feature] layout; 2 MB contiguous chunk DMAs
    (2048 tokens, partition p holds 16 consecutive token rows).
  - Per 128-token sub-chunk: 4 PE transposes (fp16 = 1 cyc/row) put features
    on partitions; ACT copies xT PSUM->SBUF; 4 fp16 matmuls (1 cyc/row,
    lhsT=xT stationary, rhs=block-diag weights) write a [token, 512] PSUM
    bank; DVE adds the (broadcast) bias during the PSUM->SBUF fp16
    downconvert copy.
  - 2 MB contiguous chunk DMA out (fp16) on the second HWDGE ring.
Engine busy/core: DMA ~92 us (the roofline: 32 MB at ~350 GB/s), DVE ~84 us,
PE ~55 us, ACT ~51 us; measured ~95-100 us vs 1478 us for the fp32 baseline.

The older fp32 builders (build_nc, build_nc_tok, build_nc_ht) are kept below
for reference/bisection.
"""

import numpy as np

import concourse.bass as bass
import concourse.bacc as bacc
import concourse.mybir as mybir
import concourse.tile as tile

B, S = 4, 4096
IN_F = OUT_F = 4096
NB, IPB, OPB = 128, 32, 32
NCORES = 8
BPC = NB // NCORES            # blocks per core = 16
FPC = BPC * IPB               # features per core = 512
TOK = B * S                   # tokens = 16384
GROUPS = FPC // 128           # 128-feature groups per core = 4
BLOCKS_PER_GROUP = 128 // IPB  # 4

F32 = mybir.dt.float32
F16 = mybir.dt.float16


def build_nc_v2(
    loop_reps: int = 1,
    ch: int = 2048,            # tokens per chunk
    dve_cols: int = 288,       # PSUM->SBUF columns copied by DVE (rest on ACT)
    bias_pe: bool = True,      # add bias via K=1 ones-matmul on PE
    psum_bufs: int = 5,
    mode: str = "dmat",        # "dmat" (DMA-transpose in) | "pet" (PE transpose)
    xbufs: int = 2,
    obufs: int = 2,
    variant: str = "full",     # full | dma (DMA-only: in->out passthrough)
    pair: int = 1,             # subs per PSUM tile / DVE add (1 or 2)
    psx_bufs: int = 3,
):
    """fp16 redesign, block-sharded (16 blocks = 512 features per core).

    mode="dmat": host packs x fp16 as [nchunk, 4*ch, 128] (group-major,
    token-permuted); one 16-bit xbar DMA-transpose per chunk lands features
    on partitions directly -- no PE transpose, no xT PSUM round trip.
    Per 128-token sub-chunk: 1 bias matmul (K=1 ones x bias row, start=True)
    + 4 fp16 block-diag matmuls (1 cyc/row) accumulate into a PSUM bank in
    [token, feature] layout; DVE/ACT split the PSUM->SBUF fp16 downconvert
    copy. Input and output DMAs are fully contiguous.

    mode="pet": natural-layout fp16 input; PE transposes (1 cyc/row) feed
    the same matmul pipeline; ACT does the xT PSUM->SBUF copy, DVE the
    output copy.
    """
    assert TOK % ch == 0 and ch % 128 == 0
    nchunk = TOK // ch
    sub = ch // 128
    if not bias_pe:
        dve_cols = FPC

    nc = bacc.Bacc(
        "TRN2", target_bir_lowering=False, debug=False, num_devices=NCORES
    )
    if mode == "dmat":
        xg = nc.dram_tensor("xg", [nchunk, GROUPS * ch, 128], F16,
                            kind="ExternalInput").ap()
    else:
        xs = nc.dram_tensor("xs", [TOK, FPC], F16, kind="ExternalInput").ap()
        xs3 = xs.rearrange("(c p a) f -> c p (a f)", p=128, a=sub)
        idn = nc.dram_tensor("idn", [128, 128], F16, kind="ExternalInput").ap()
    wt_d = nc.dram_tensor("wt", [128, FPC], F16, kind="ExternalInput").ap()
    b1_d = nc.dram_tensor("b1", [1, FPC], F16, kind="ExternalInput").ap()
    on_d = nc.dram_tensor("on", [1, 128], F16, kind="ExternalInput").ap()
    bt_d = nc.dram_tensor("bt", [128, FPC], F32, kind="ExternalInput").ap()
    out = nc.dram_tensor("out", [TOK, FPC], F16, kind="ExternalOutput").ap()
    out3 = out.rearrange("(c p a) f -> c p (a f)", p=128, a=sub)

    import contextlib

    with tile.TileContext(nc) as tc:
        with (
            tc.tile_pool(name="const", bufs=1) as cpool,
            tc.tile_pool(name="xin", bufs=xbufs) as xpool,
            tc.tile_pool(name="oout", bufs=obufs) as opool,
            tc.tile_pool(name="xt", bufs=3) as xtpool,
            tc.tile_pool(name="ps", bufs=psum_bufs, space="PSUM") as pspool,
            tc.tile_pool(name="psx", bufs=psx_bufs, space="PSUM") as psxpool,
        ):
            wt = cpool.tile([128, FPC], F16)
            nc.sync.dma_start(out=wt[:], in_=wt_d)
            b1t = cpool.tile([1, FPC], F16)
            nc.sync.dma_start(out=b1t[:], in_=b1_d)
            ot1 = cpool.tile([1, 128], F16)
            nc.sync.dma_start(out=ot1[:], in_=on_d)
            if not bias_pe:
                bt = cpool.tile([128, pair * FPC], F32)
                for h in range(pair):
                    nc.sync.dma_start(out=bt[:, bass.ts(h, FPC)], in_=bt_d)
            if mode == "pet":
                it = cpool.tile([128, 128], F16)
                nc.sync.dma_start(out=it[:], in_=idn)

            loop_ctx = (
                tc.For_i(
                    0, loop_reps, 1,
                    hint_engines=(mybir.EngineType.PE, mybir.EngineType.DVE),
                )
                if loop_reps > 1
                else contextlib.nullcontext()
            )
            with loop_ctx:
                for c in range(nchunk):
                    if mode == "dmat":
                        xt = xpool.tile([128, GROUPS * ch], F16)
                        nc.sync.dma_start(out=xt[:], in_=xg[c], transpose=True)
                    else:
                        x_in = xpool.tile([128, sub * FPC], F16)
                        nc.sync.dma_start(out=x_in[:], in_=xs3[c])
                    if variant == "dma":
                        src = xt if mode == "dmat" else x_in
                        nc.scalar.dma_start(out=out3[c], in_=src[:])
                        continue
                    ot = opool.tile([128, sub * FPC], F16)
                    for s0 in range(0, sub, pair):
                        ps = pspool.tile([128, pair * FPC], F32)
                        for h in range(pair):
                            s = s0 + h
                            if mode == "pet":
                                xt_ps = psxpool.tile([128, FPC], F16)
                                for g in range(GROUPS):
                                    nc.tensor.transpose(
                                        xt_ps[:, bass.ts(g, 128)],
                                        x_in[:, s * FPC + g * 128:
                                             s * FPC + (g + 1) * 128],
                                        it[:],
                                    )
                                xt_sb = xtpool.tile([128, FPC], F16)
                                nc.scalar.copy(xt_sb[:], xt_ps[:])

                            def lhs(g):
                                if mode == "dmat":
                                    return xt[:, g * ch + s * 128:
                                              g * ch + (s + 1) * 128]
                                return xt_sb[:, bass.ts(g, 128)]

                            if bias_pe:
                                nc.tensor.matmul(
                                    ps[:, bass.ts(h, FPC)],
                                    lhsT=ot1[:], rhs=b1t[:],
                                    start=True, stop=False,
                                )
                            for g in range(GROUPS):
                                nc.tensor.matmul(
                                    ps[:, h * FPC + g * 128:
                                       h * FPC + (g + 1) * 128],
                                    lhsT=lhs(g),
                                    rhs=wt[:, bass.ts(g, 128)],
                                    start=not bias_pe,
                                    stop=True,
                                )
                            if bias_pe:
                                if dve_cols:
                                    nc.vector.tensor_copy(
                                        ot[:, s * FPC: s * FPC + dve_cols],
                                        ps[:, h * FPC: h * FPC + dve_cols],
                                    )
                                if dve_cols < FPC:
                                    nc.scalar.copy(
                                        ot[:, s * FPC + dve_cols:
                                           (s + 1) * FPC],
                                        ps[:, h * FPC + dve_cols:
                                           (h + 1) * FPC],
                                    )
                        if not bias_pe:
                            nc.vector.tensor_add(
                                ot[:, s0 * FPC: (s0 + pair) * FPC],
                                ps[:], bt[:],
                            )
                    nc.scalar.dma_start(out=out3[c], in_=ot[:])
    nc.compile()
    return nc


def prep_in_maps_v2(x, weight, bias, ch: int = 2048, mode: str = "dmat"):
    """Per-core fp16 input maps for build_nc_v2."""
    x2 = np.asarray(x, np.float32).reshape(TOK, IN_F)
    w = np.asarray(weight, np.float32)
    b = np.asarray(bias, np.float32)
    nchunk = TOK // ch
    sub = ch // 128
    ones = np.ones((1, 128), np.float16)
    ident = np.eye(128, dtype=np.float16)
    maps = []
    for m in range(NCORES):
        xs = x2[:, m * FPC:(m + 1) * FPC].astype(np.float16)   # [TOK, 512]
        wm = w[m * BPC:(m + 1) * BPC]                          # [16, 32, 32]
        wg = np.zeros((128, FPC), np.float16)
        for g in range(GROUPS):
            for a in range(BLOCKS_PER_GROUP):
                wg[a * 32:(a + 1) * 32,
                   g * 128 + a * 32: g * 128 + (a + 1) * 32] = wm[4 * g + a]
        b1 = b[m * BPC:(m + 1) * BPC].reshape(1, FPC).astype(np.float16)
        btm = np.ascontiguousarray(
            np.broadcast_to(b[m * BPC:(m + 1) * BPC].reshape(FPC), (128, FPC))
        ).astype(np.float32)
        mp = {"wt": wg, "b1": b1, "on": ones, "bt": btm}
        if mode == "dmat":
            # row (c, g, s*128+p) of xg = features [g*128:(g+1)*128] of
            # original token c*ch + p*sub + s
            xgm = xs.reshape(nchunk, 128, sub, GROUPS, 128)    # [c,p,s,g,u]
            xgm = np.ascontiguousarray(xgm.transpose(0, 3, 2, 1, 4))
            mp["xg"] = xgm.reshape(nchunk, GROUPS * ch, 128)
        else:
            mp["xs"] = xs
            mp["idn"] = ident
        maps.append(mp)
    return maps


def kernel_v2(inputs, ch: int = 2048, mode: str = "dmat", **bkw) -> np.ndarray:
    from concourse.bass_utils import run_bass_kernel_spmd

    nc = build_nc_v2(ch=ch, mode=mode, **bkw)
    in_maps = prep_in_maps_v2(inputs["x"], inputs["weight"], inputs["bias"],
                              ch=ch, mode=mode)
    res = run_bass_kernel_spmd(nc, in_maps, core_ids=list(range(NCORES)))
    outs = [res.results[m]["out"].astype(np.float32) for m in range(NCORES)]
    full = np.concatenate(outs, axis=1)           # [16384, 4096]
    return full.reshape(B, S, OUT_F)


def build_nc(
    tok: int = TOK,
    chunk_tok: int = 1024,
    reps: int = 1,
    loop_reps: int = 1,
    use_f32r: bool = False,
    variant: str = "full",      # full | dma | nomm | notr  (bisection variants)
    copy_engine: str = "vector",  # engine for the xT PSUM->SBUF copy
):
    """Build the per-core Bass program (SPMD: same program, per-core data).

    reps: python-unrolled repetitions of the whole pass (for timing).
    loop_reps: hardware For_i loop repetitions of the whole pass (for timing
    with constant instruction count).
    use_f32r: stream operands as float32r (same bits as fp32, faster PE
    streaming mode) and run the matmuls as zero-padded pairs with a 256-wide
    moving dim, where f32r hits 1 cycle/row instead of fp32's 4.
    """
    assert tok % chunk_tok == 0 and chunk_tok % 128 == 0
    nchunk = tok // chunk_tok
    sub = chunk_tok // 128     # 128-token sub-chunks per chunk
    XD = mybir.dt.float32r if use_f32r else F32

    nc = bacc.Bacc(
        "TRN2", target_bir_lowering=False, debug=False, num_devices=NCORES
    )
    xs = nc.dram_tensor("xs", [tok, FPC], XD, kind="ExternalInput").ap()
    if use_f32r:
        wpad = nc.dram_tensor(
            "wpad", [GROUPS, 128, 256], XD, kind="ExternalInput"
        ).ap()
    else:
        wbd = nc.dram_tensor("wbd", [GROUPS, 128, 128], F32, kind="ExternalInput").ap()
    bb = nc.dram_tensor("bb", [128, FPC], F32, kind="ExternalInput").ap()
    idn = nc.dram_tensor("idn", [128, 128], XD, kind="ExternalInput").ap()
    out = nc.dram_tensor("out", [tok, FPC], F32, kind="ExternalOutput").ap()

    xs3 = xs.rearrange("(c a p) f -> c p a f", a=sub, p=128)
    out3 = out.rearrange("(c a p) f -> c p a f", a=sub, p=128)

    with tile.TileContext(nc) as tc:
        with (
            tc.tile_pool(name="const", bufs=1) as cpool,
            tc.tile_pool(name="xin", bufs=2) as xpool,
            tc.tile_pool(name="oout", bufs=2) as opool,
            tc.tile_pool(name="xt", bufs=3) as xtpool,
            tc.tile_pool(name="ps", bufs=2, space="PSUM") as pspool,
        ):
            if use_f32r:
                wt = cpool.tile([128, GROUPS * 256], XD)
                nc.sync.dma_start(
                    out=wt[:].rearrange("p (g m) -> p g m", g=GROUPS),
                    in_=wpad.rearrange("g k m -> k g m"),
                )
            else:
                wt = cpool.tile([128, GROUPS * 128], F32)
                nc.sync.dma_start(
                    out=wt[:].rearrange("p (g m) -> p g m", g=GROUPS),
                    in_=wbd.rearrange("g k m -> k g m"),
                )
            bt = cpool.tile([128, FPC], F32)
            nc.sync.dma_start(out=bt[:], in_=bb)
            it = cpool.tile([128, 128], XD)
            nc.sync.dma_start(out=it[:], in_=idn)

            import contextlib

            loop_ctx = (
                tc.For_i(
                    0,
                    loop_reps,
                    1,
                    hint_engines=(mybir.EngineType.PE, mybir.EngineType.Activation),
                )
                if loop_reps > 1
                else contextlib.nullcontext()
            )
            with loop_ctx:
                for _ in range(reps):
                    for c in range(nchunk):
                        x_in = xpool.tile([128, sub * FPC], XD)
                        nc.sync.dma_start(
                            out=x_in[:].rearrange("p (a f) -> p a f", a=sub),
                            in_=xs3[c],
                        )
                        if variant == "dma":
                            nc.scalar.dma_start(
                                out=out3[c],
                                in_=x_in[:].rearrange("p (a f) -> p a f", a=sub),
                            )
                            continue
                        cp_fn = (
                            nc.scalar.copy
                            if copy_engine == "scalar"
                            else nc.vector.tensor_copy
                        )
                        ot = opool.tile([128, sub * FPC], F32)
                        for s in range(sub):
                            if variant != "notr":
                                xT_ps = pspool.tile([128, FPC], XD)
                                for g in range(GROUPS):
                                    nc.tensor.transpose(
                                        xT_ps[:, bass.ts(g, 128)],
                                        x_in[
                                            :,
                                            s * FPC + g * 128 : s * FPC + (g + 1) * 128,
                                        ],
                                        it[:],
                                    )
                                xT_sb = xtpool.tile([128, FPC], XD)
                                cp_fn(xT_sb[:], xT_ps[:])
                            else:
                                xT_sb = x_in[:, bass.ts(s, FPC)]
                            if variant == "nomm":
                                nc.vector.tensor_add(
                                    ot[:, bass.ts(s, FPC)], xT_ps[:], bt[:]
                                )
                                continue
                            o_ps = pspool.tile([128, FPC], F32)
                            if use_f32r:
                                for p in range(GROUPS // 2):
                                    for h in range(2):
                                        nc.tensor.matmul(
                                            o_ps[:, bass.ts(p, 256)],
                                            lhsT=xT_sb[:, bass.ts(2 * p + h, 128)],
                                            rhs=wt[:, bass.ts(2 * p + h, 256)],
                                            start=(h == 0),
                                            stop=(h == 1),
                                        )
                            else:
                                for g in range(GROUPS):
                                    nc.tensor.matmul(
                                        o_ps[:, bass.ts(g, 128)],
                                        lhsT=xT_sb[:, bass.ts(g, 128)],
                                        rhs=wt[:, bass.ts(g, 128)],
                                        start=True,
                                        stop=True,
                                    )
                            nc.vector.tensor_add(
                                ot[:, bass.ts(s, FPC)], o_ps[:], bt[:]
                            )
                        nc.scalar.dma_start(
                            out=out3[c],
                            in_=ot[:].rearrange("p (a f) -> p a f", a=sub),
                        )
    nc.compile()
    return nc


def build_nc_tok(
    tpc: int = TOK // NCORES,
    loop_reps: int = 1,
    use_f32r: bool = False,
    qf: int = 1024,             # features per PSUM quarter (multiple of 256)
    psum_bufs: int = 2,
    variant: str = "full",      # full | dma
    mm_transpose_mode: bool = False,  # run matmuls with is_transpose=True
):
    """Token-sharded per-core program: each core owns tpc tokens x all 4096
    features. DMA is fully contiguous (16 KB per partition per transfer)."""
    assert tpc % 128 == 0
    nsub = tpc // 128
    ngrp = IN_F // 128          # 32 groups of 128 features
    nq = IN_F // qf             # PSUM quarters per sub-chunk
    gq = qf // 128              # groups per quarter
    XD = mybir.dt.float32r if use_f32r else F32

    nc = bacc.Bacc(
        "TRN2", target_bir_lowering=False, debug=False, num_devices=NCORES
    )
    xs = nc.dram_tensor("xs", [tpc, IN_F], XD, kind="ExternalInput").ap()
    if use_f32r:
        wpad = nc.dram_tensor(
            "wpad", [ngrp, 128, 256], XD, kind="ExternalInput"
        ).ap()
    else:
        wbd = nc.dram_tensor("wbd", [ngrp, 128, 128], F32, kind="ExternalInput").ap()
    bb = nc.dram_tensor("bb", [128, IN_F], F32, kind="ExternalInput").ap()
    idn = nc.dram_tensor("idn", [128, 128], XD, kind="ExternalInput").ap()
    out = nc.dram_tensor("out", [tpc, IN_F], F32, kind="ExternalOutput").ap()

    xs2 = xs.rearrange("(c p) f -> c p f", p=128)
    out2 = out.rearrange("(c p) f -> c p f", p=128)

    with tile.TileContext(nc) as tc:
        with (
            tc.tile_pool(name="const", bufs=1) as cpool,
            tc.tile_pool(name="xin", bufs=3) as xpool,
            tc.tile_pool(name="oout", bufs=3) as opool,
            tc.tile_pool(name="xt", bufs=3) as xtpool,
            tc.tile_pool(name="ps", bufs=psum_bufs, space="PSUM") as pspool,
        ):
            if use_f32r:
                wt = cpool.tile([128, ngrp * 256], XD)
                nc.sync.dma_start(
                    out=wt[:].rearrange("p (g m) -> p g m", g=ngrp),
                    in_=wpad.rearrange("g k m -> k g m"),
                )
            else:
                wt = cpool.tile([128, ngrp * 128], F32)
                nc.sync.dma_start(
                    out=wt[:].rearrange("p (g m) -> p g m", g=ngrp),
                    in_=wbd.rearrange("g k m -> k g m"),
                )
            bt = cpool.tile([128, IN_F], F32)
            nc.sync.dma_start(out=bt[:], in_=bb)
            it = cpool.tile([128, 128], XD)
            nc.sync.dma_start(out=it[:], in_=idn)

            import contextlib

            loop_ctx = (
                tc.For_i(
                    0,
                    loop_reps,
                    1,
                    hint_engines=(mybir.EngineType.PE, mybir.EngineType.DVE),
                )
                if loop_reps > 1
                else contextlib.nullcontext()
            )
            with loop_ctx:
                for c in range(nsub):
                    x_in = xpool.tile([128, IN_F], XD)
                    nc.sync.dma_start(out=x_in[:], in_=xs2[c])
                    if variant == "dma":
                        nc.scalar.dma_start(out=out2[c], in_=x_in[:])
                        continue
                    ot = opool.tile([128, IN_F], F32)
                    for q in range(nq):
                        xT_ps = pspool.tile([128, qf], XD)
                        for g in range(gq):
                            nc.tensor.transpose(
                                xT_ps[:, bass.ts(g, 128)],
                                x_in[:, q * qf + g * 128 : q * qf + (g + 1) * 128],
                                it[:],
                            )
                        xT_sb = xtpool.tile([128, qf], XD)
                        nc.vector.tensor_copy(xT_sb[:], xT_ps[:])
                        o_ps = pspool.tile([128, qf], F32)
                        if use_f32r:
                            for p in range(gq // 2):
                                for h in range(2):
                                    nc.tensor.matmul(
                                        o_ps[:, bass.ts(p, 256)],
                                        lhsT=xT_sb[:, bass.ts(2 * p + h, 128)],
                                        rhs=wt[
                                            :,
                                            (q * gq + 2 * p + h)
                                            * 256 : (q * gq + 2 * p + h + 1)
                                            * 256,
                                        ],
                                        start=(h == 0),
                                        stop=(h == 1),
                                    )
                        else:
                            for g in range(gq):
                                nc.tensor.matmul(
                                    o_ps[:, bass.ts(g, 128)],
                                    lhsT=xT_sb[:, bass.ts(g, 128)],
                                    rhs=wt[:, bass.ts(q * gq + g, 128)],
                                    start=True,
                                    stop=True,
                                    is_transpose=mm_transpose_mode or None,
                                )
                        nc.vector.tensor_add(
                            ot[:, bass.ts(q, qf)], o_ps[:], bt[:, bass.ts(q, qf)]
                        )
                    nc.scalar.dma_start(out=out2[c], in_=ot[:])
    nc.compile()
    return nc


def build_nc_ht(
    tpc: int = TOK // NCORES,
    loop_reps: int = 1,
    win_tok: int = 256,         # tokens per input window (one 4MB DMA each)
    psum_bufs: int = 6,
    use_f32r: bool = False,
):
    """Host-transposed per-core program: x arrives feature-major [4096, tpc],
    so features land on partitions straight from DMA — no on-chip transpose,
    no PSUM round-trip for inputs. Token-sharded across cores."""
    assert tpc % win_tok == 0 and win_tok % 128 == 0
    nwin = tpc // win_tok
    tc_per_win = win_tok // 128
    ngrp = IN_F // 128          # 32
    XD = mybir.dt.float32r if use_f32r else F32

    nc = bacc.Bacc(
        "TRN2", target_bir_lowering=False, debug=False, num_devices=NCORES
    )
    xt = nc.dram_tensor("xt", [IN_F, tpc], XD, kind="ExternalInput").ap()
    if use_f32r:
        wpad = nc.dram_tensor(
            "wpad", [ngrp, 128, 256], XD, kind="ExternalInput"
        ).ap()
    else:
        wbd = nc.dram_tensor("wbd", [ngrp, 128, 128], F32, kind="ExternalInput").ap()
    bb = nc.dram_tensor("bb", [128, IN_F], F32, kind="ExternalInput").ap()
    out = nc.dram_tensor("out", [tpc, IN_F], F32, kind="ExternalOutput").ap()

    xt4 = xt.rearrange("(g p) t -> p g t", g=ngrp, p=128)  # [128, 32, tpc]
    out2 = out.rearrange("(c p) f -> c p f", p=128)

    with tile.TileContext(nc) as tc:
        with (
            tc.tile_pool(name="const", bufs=1) as cpool,
            tc.tile_pool(name="xin", bufs=2) as xpool,
            tc.tile_pool(name="oout", bufs=2) as opool,
            tc.tile_pool(name="ps", bufs=psum_bufs, space="PSUM") as pspool,
        ):
            if use_f32r:
                wt = cpool.tile([128, ngrp * 256], XD)
                nc.sync.dma_start(
                    out=wt[:].rearrange("p (g m) -> p g m", g=ngrp),
                    in_=wpad.rearrange("g k m -> k g m"),
                )
            else:
                wt = cpool.tile([128, ngrp * 128], F32)
                nc.sync.dma_start(
                    out=wt[:].rearrange("p (g m) -> p g m", g=ngrp),
                    in_=wbd.rearrange("g k m -> k g m"),
                )
            bt = cpool.tile([128, IN_F], F32)
            nc.sync.dma_start(out=bt[:], in_=bb)

            import contextlib

            loop_ctx = (
                tc.For_i(
                    0,
                    loop_reps,
                    1,
                    hint_engines=(mybir.EngineType.PE, mybir.EngineType.DVE),
                )
                if loop_reps > 1
                else contextlib.nullcontext()
            )
            with loop_ctx:
                for w in range(nwin):
                    xw = xpool.tile([128, ngrp * win_tok], XD)
                    nc.sync.dma_start(
                        out=xw[:].rearrange("p (g t) -> p g t", g=ngrp),
                        in_=xt4[:, :, w * win_tok : (w + 1) * win_tok],
                    )
                    for tci in range(tc_per_win):
                        ot = opool.tile([128, IN_F], F32)
                        for q in range(IN_F // 512):
                            o_ps = pspool.tile([128, 512], F32)
                            if use_f32r:
                                for p in range(2):
                                    for h in range(2):
                                        g = q * 4 + 2 * p + h
                                        nc.tensor.matmul(
                                            o_ps[:, bass.ts(p, 256)],
                                            lhsT=xw[
                                                :,
                                                g * win_tok
                                                + tci * 128 : g * win_tok
                                                + tci * 128
                                                + 128,
                                            ],
                                            rhs=wt[:, bass.ts(g, 256)],
                                            start=(h == 0),
                                            stop=(h == 1),
                                        )
                            else:
                                for j in range(4):
                                    g = q * 4 + j
                                    nc.tensor.matmul(
                                        o_ps[:, bass.ts(j, 128)],
                                        lhsT=xw[
                                            :,
                                            g * win_tok
                                            + tci * 128 : g * win_tok
                                            + tci * 128
                                            + 128,
                                        ],
                                        rhs=wt[:, bass.ts(g, 128)],
                                        start=True,
                                        stop=True,
                                    )
                            nc.vector.tensor_add(
                                ot[:, bass.ts(q, 512)],
                                o_ps[:],
                                bt[:, bass.ts(q, 512)],
                            )
                        nc.scalar.dma_start(
                            out=out2[w * tc_per_win + tci], in_=ot[:]
                        )
    nc.compile()
    return nc


def prep_in_maps_ht(x, weight, bias, use_f32r: bool = False):
    """Host-transposed inputs: per-core feature-major x slice."""
    x = np.asarray(x, dtype=np.float32).reshape(-1, IN_F)
    weight = np.asarray(weight, dtype=np.float32)
    bias = np.asarray(bias, dtype=np.float32)
    tpc = x.shape[0] // NCORES

    ngrp = IN_F // 128
    bpg = 128 // IPB
    wg = np.zeros((ngrp, 128, 128), np.float32)
    for g in range(ngrp):
        for a in range(bpg):
            wg[g, 32 * a : 32 * a + 32, 32 * a : 32 * a + 32] = weight[bpg * g + a]
    bbm = np.ascontiguousarray(np.broadcast_to(bias.reshape(IN_F), (128, IN_F)))
    maps = []
    for m in range(NCORES):
        xtm = np.ascontiguousarray(x[m * tpc : (m + 1) * tpc].T)
        mp = {"xt": xtm, "bb": bbm}
        if use_f32r:
            wp = np.zeros((ngrp, 128, 256), np.float32)
            for qq in range(ngrp):
                h = qq % 2
                wp[qq, :, 128 * h : 128 * h + 128] = wg[qq]
            mp["wpad"] = wp
        else:
            mp["wbd"] = wg
        maps.append(mp)
    return maps


def prep_in_maps_tok(x, weight, bias):
    """Token-sharded inputs: per-core contiguous token slice, shared weights."""
    x = np.ascontiguousarray(np.asarray(x, dtype=np.float32).reshape(-1, IN_F))
    weight = np.asarray(weight, dtype=np.float32)
    bias = np.asarray(bias, dtype=np.float32)
    ident = np.eye(128, dtype=np.float32)
    tpc = x.shape[0] // NCORES

    ngrp = IN_F // 128
    bpg = 128 // IPB            # blocks per 128-feature group = 4
    wg = np.zeros((ngrp, 128, 128), np.float32)
    for g in range(ngrp):
        for a in range(bpg):
            wg[g, 32 * a : 32 * a + 32, 32 * a : 32 * a + 32] = weight[bpg * g + a]
    wp = np.zeros((ngrp, 128, 256), np.float32)
    for qq in range(ngrp):
        h = qq % 2
        wp[qq, :, 128 * h : 128 * h + 128] = wg[qq]
    bbm = np.ascontiguousarray(
        np.broadcast_to(bias.reshape(IN_F), (128, IN_F))
    )
    return [
        {
            "xs": x[m * tpc : (m + 1) * tpc],
            "wbd": wg,
            "wpad": wp,
            "bb": bbm,
            "idn": ident,
        }
        for m in range(NCORES)
    ]


def prep_in_maps(x, weight, bias, tok: int = TOK):
    """Split full inputs into 8 per-core input maps (host-side numpy)."""
    x = np.asarray(x, dtype=np.float32).reshape(-1, IN_F)[:tok]
    weight = np.asarray(weight, dtype=np.float32)
    bias = np.asarray(bias, dtype=np.float32)
    ident = np.eye(128, dtype=np.float32)

    in_maps = []
    for m in range(NCORES):
        xs = np.ascontiguousarray(x[:, m * FPC : (m + 1) * FPC])
        wm = weight[m * BPC : (m + 1) * BPC]          # [16, 32, 32]
        wg = np.zeros((GROUPS, 128, 128), np.float32)
        for g in range(GROUPS):
            for a in range(BLOCKS_PER_GROUP):
                wg[g, 32 * a : 32 * a + 32, 32 * a : 32 * a + 32] = wm[
                    BLOCKS_PER_GROUP * g + a
                ]
        # zero-padded pairs for the f32r N=256 matmul path: entry q = 2p+h
        # holds group (2p+h)'s weights in column half h, zeros in the other.
        wp = np.zeros((GROUPS, 128, 256), np.float32)
        for q in range(GROUPS):
            h = q % 2
            wp[q, :, 128 * h : 128 * h + 128] = wg[q]
        bm = bias[m * BPC : (m + 1) * BPC].reshape(FPC)
        bbm = np.ascontiguousarray(np.broadcast_to(bm, (128, FPC)))
        in_maps.append({"xs": xs, "wbd": wg, "wpad": wp, "bb": bbm, "idn": ident})
    return in_maps


# Best measured config (pet = PE-transpose path, fp16 compute, DVE bias-add,
# 3x double-buffering): ~94.5 us/core on HW, at the 32 MB/core DMA floor.
BEST = dict(ch=2048, mode="pet", bias_pe=False, xbufs=3, obufs=3)


def kernel(**inputs) -> np.ndarray:
    from concourse.bass_utils import run_bass_kernel_spmd

    nc = build_nc_v2(**BEST)
    in_maps = prep_in_maps_v2(inputs["x"], inputs["weight"], inputs["bias"],
                              ch=BEST["ch"], mode=BEST["mode"])
    res = run_bass_kernel_spmd(nc, in_maps, core_ids=list(range(NCORES)))
    outs = [res.results[m]["out"].astype(np.float32) for m in range(NCORES)]
    full = np.concatenate(outs, axis=1)           # [16384, 4096]
    return full.reshape(B, S, OUT_F)



# revision 18
# speedup vs baseline: 1.4756x; 1.1767x over previous
"""Grouped (block-diagonal) linear kernel for Trainium2, 8 NeuronCores.

Problem: x [4, 4096, 4096] f32, weight [128, 32, 32], bias [128, 32].
out[b,s,n,o] = sum_i x[b,s,n*32+i] * weight[n,i,o] + bias[n,o], flattened back
to [4, 4096, 4096].

Sharding: the 128 blocks are split across 8 cores (16 blocks = 512 features
per core). Each core reads its own 512-column slice of x and writes the
matching 512-column slice of the output; results are concatenated on host.

Current design (build_nc_v2, mode="pet"): fp16 compute, 16 MB in + 16 MB out
per core (host converts x to fp16 and upconverts the fp16 result; matmul
accumulation stays fp32 in PSUM, rel err ~6e-4 vs the 2e-2 gate):
  - x fp16 in natural [token, feature] layout; 2 MB contiguous chunk DMAs
    (2048 tokens, partition p holds 16 consecutive token rows).
  - Per 128-token sub-chunk: 4 PE transposes (fp16 = 1 cyc/row) put features
    on partitions; ACT copies xT PSUM->SBUF; 4 fp16 matmuls (1 cyc/row,
    lhsT=xT stationary, rhs=block-diag weights) write a [token, 512] PSUM
    bank; DVE adds the (broadcast) bias during the PSUM->SBUF fp16
    downconvert copy.
  - 2 MB contiguous chunk DMA out (fp16) on the second HWDGE ring.
Engine busy/core: DMA ~92 us (the roofline: 32 MB at ~350 GB/s), DVE ~84 us,
PE ~55 us, ACT ~51 us; measured ~95-100 us vs 1478 us for the fp32 baseline.

The older fp32 builders (build_nc, build_nc_tok, build_nc_ht) are kept below
for reference/bisection.
"""

import numpy as np

import concourse.bass as bass
import concourse.bacc as bacc
import concourse.mybir as mybir
import concourse.tile as tile

B, S = 4, 4096
IN_F = OUT_F = 4096
NB, IPB, OPB = 128, 32, 32
NCORES = 8
BPC = NB // NCORES            # blocks per core = 16
FPC = BPC * IPB               # features per core = 512
TOK = B * S                   # tokens = 16384
GROUPS = FPC // 128           # 128-feature groups per core = 4
BLOCKS_PER_GROUP = 128 // IPB  # 4

F32 = mybir.dt.float32
F16 = mybir.dt.float16


def build_nc_v2(
    loop_reps: int = 1,
    ch: int = 2048,            # tokens per chunk
    dve_cols: int = 288,       # PSUM->SBUF columns copied by DVE (rest on ACT)
    bias_pe: bool = True,      # add bias via K=1 ones-matmul on PE
    psum_bufs: int = 5,
    mode: str = "dmat",        # "dmat" (DMA-transpose in) | "pet" (PE transpose)
    xbufs: int = 2,
    obufs: int = 2,
    variant: str = "full",     # full | dma (DMA-only: in->out passthrough)
    pair: int = 1,             # subs per PSUM tile / DVE add (1 or 2)
    psx_bufs: int = 3,
    skew: int = 0,             # pet only: emit transposes `skew` subs ahead
    isplit: int = 1,           # input DMAs per chunk (skew path only)
    osplit: int = 1,           # output DMAs per chunk (skew path only)
):
    """fp16 redesign, block-sharded (16 blocks = 512 features per core).

    mode="dmat": host packs x fp16 as [nchunk, 4*ch, 128] (group-major,
    token-permuted); one 16-bit xbar DMA-transpose per chunk lands features
    on partitions directly -- no PE transpose, no xT PSUM round trip.
    Per 128-token sub-chunk: 1 bias matmul (K=1 ones x bias row, start=True)
    + 4 fp16 block-diag matmuls (1 cyc/row) accumulate into a PSUM bank in
    [token, feature] layout; DVE/ACT split the PSUM->SBUF fp16 downconvert
    copy. Input and output DMAs are fully contiguous.

    mode="pet": natural-layout fp16 input; PE transposes (1 cyc/row) feed
    the same matmul pipeline; ACT does the xT PSUM->SBUF copy, DVE the
    output copy.
    """
    assert TOK % ch == 0 and ch % 128 == 0
    nchunk = TOK // ch
    sub = ch // 128
    if not bias_pe:
        dve_cols = FPC

    nc = bacc.Bacc(
        "TRN2", target_bir_lowering=False, debug=False, num_devices=NCORES
    )
    if mode == "dmat":
        xg = nc.dram_tensor("xg", [nchunk, GROUPS * ch, 128], F16,
                            kind="ExternalInput").ap()
    else:
        xs = nc.dram_tensor("xs", [TOK, FPC], F16, kind="ExternalInput").ap()
        xs3 = xs.rearrange("(c p a) f -> c p (a f)", p=128, a=sub)
        idn = nc.dram_tensor("idn", [128, 128], F16, kind="ExternalInput").ap()
    wt_d = nc.dram_tensor("wt", [128, FPC], F16, kind="ExternalInput").ap()
    b1_d = nc.dram_tensor("b1", [1, FPC], F16, kind="ExternalInput").ap()
    on_d = nc.dram_tensor("on", [1, 128], F16, kind="ExternalInput").ap()
    bt_d = nc.dram_tensor("bt", [128, FPC], F32, kind="ExternalInput").ap()
    out = nc.dram_tensor("out", [TOK, FPC], F16, kind="ExternalOutput").ap()
    out3 = out.rearrange("(c p a) f -> c p (a f)", p=128, a=sub)

    import contextlib

    with tile.TileContext(nc) as tc:
        with (
            tc.tile_pool(name="const", bufs=1) as cpool,
            tc.tile_pool(name="xin", bufs=xbufs) as xpool,
            tc.tile_pool(name="oout", bufs=obufs) as opool,
            tc.tile_pool(name="xt", bufs=3) as xtpool,
            tc.tile_pool(name="ps", bufs=psum_bufs, space="PSUM") as pspool,
            tc.tile_pool(name="psx", bufs=psx_bufs, space="PSUM") as psxpool,
        ):
            wt = cpool.tile([128, FPC], F16)
            nc.sync.dma_start(out=wt[:], in_=wt_d)
            b1t = cpool.tile([1, FPC], F16)
            nc.sync.dma_start(out=b1t[:], in_=b1_d)
            ot1 = cpool.tile([1, 128], F16)
            nc.sync.dma_start(out=ot1[:], in_=on_d)
            if not bias_pe:
                bt = cpool.tile([128, pair * FPC], F32)
                for h in range(pair):
                    nc.sync.dma_start(out=bt[:, bass.ts(h, FPC)], in_=bt_d)
            if mode == "pet":
                it = cpool.tile([128, 128], F16)
                nc.sync.dma_start(out=it[:], in_=idn)

            loop_ctx = (
                tc.For_i(
                    0, loop_reps, 1,
                    hint_engines=(mybir.EngineType.PE, mybir.EngineType.DVE),
                )
                if loop_reps > 1
                else contextlib.nullcontext()
            )
            with loop_ctx:
                for c in range(nchunk):
                    if mode == "dmat":
                        xt = xpool.tile([128, GROUPS * ch], F16)
                        nc.sync.dma_start(out=xt[:], in_=xg[c], transpose=True)
                    else:
                        x_in = xpool.tile([128, sub * FPC], F16)
                        iw = sub * FPC // isplit
                        for i in range(isplit):
                            nc.sync.dma_start(
                                out=x_in[:, i * iw:(i + 1) * iw],
                                in_=xs3[c][:, i * iw:(i + 1) * iw])
                    if variant == "dma":
                        src = xt if mode == "dmat" else x_in
                        nc.scalar.dma_start(out=out3[c], in_=src[:])
                        continue
                    ot = opool.tile([128, sub * FPC], F16)
                    if skew and mode == "pet" and pair == 1 and not bias_pe:
                        def emit_trans(s):
                            xt_ps = psxpool.tile([128, FPC], F16)
                            for g in range(GROUPS):
                                nc.tensor.transpose(
                                    xt_ps[:, bass.ts(g, 128)],
                                    x_in[:, s * FPC + g * 128:
                                         s * FPC + (g + 1) * 128],
                                    it[:],
                                )
                            xt_sb = xtpool.tile([128, FPC], F16)
                            nc.scalar.copy(xt_sb[:], xt_ps[:])
                            return xt_sb

                        def emit_mm(s, xt_sb):
                            ps = pspool.tile([128, FPC], F32)
                            for g in range(GROUPS):
                                nc.tensor.matmul(
                                    ps[:, bass.ts(g, 128)],
                                    lhsT=xt_sb[:, bass.ts(g, 128)],
                                    rhs=wt[:, bass.ts(g, 128)],
                                    start=True, stop=True,
                                )
                            nc.vector.tensor_add(
                                ot[:, bass.ts(s, FPC)], ps[:], bt[:]
                            )

                        ow = sub * FPC // osplit
                        done = 0

                        def emit_done(s):
                            nonlocal done
                            while (s + 1) * FPC >= (done + 1) * ow:
                                nc.scalar.dma_start(
                                    out=out3[c][:, done * ow:(done + 1) * ow],
                                    in_=ot[:, done * ow:(done + 1) * ow])
                                done += 1

                        pending = []
                        for s in range(sub):
                            pending.append((s, emit_trans(s)))
                            if len(pending) > skew:
                                s2, sb2 = pending.pop(0)
                                emit_mm(s2, sb2)
                                emit_done(s2)
                        for s2, sb2 in pending:
                            emit_mm(s2, sb2)
                            emit_done(s2)
                        continue
                    for s0 in range(0, sub, pair):
                        ps = pspool.tile([128, pair * FPC], F32)
                        for h in range(pair):
                            s = s0 + h
                            if mode == "pet":
                                xt_ps = psxpool.tile([128, FPC], F16)
                                for g in range(GROUPS):
                                    nc.tensor.transpose(
                                        xt_ps[:, bass.ts(g, 128)],
                                        x_in[:, s * FPC + g * 128:
                                             s * FPC + (g + 1) * 128],
                                        it[:],
                                    )
                                xt_sb = xtpool.tile([128, FPC], F16)
                                nc.scalar.copy(xt_sb[:], xt_ps[:])

                            def lhs(g):
                                if mode == "dmat":
                                    return xt[:, g * ch + s * 128:
                                              g * ch + (s + 1) * 128]
                                return xt_sb[:, bass.ts(g, 128)]

                            if bias_pe:
                                nc.tensor.matmul(
                                    ps[:, bass.ts(h, FPC)],
                                    lhsT=ot1[:], rhs=b1t[:],
                                    start=True, stop=False,
                                )
                            for g in range(GROUPS):
                                nc.tensor.matmul(
                                    ps[:, h * FPC + g * 128:
                                       h * FPC + (g + 1) * 128],
                                    lhsT=lhs(g),
                                    rhs=wt[:, bass.ts(g, 128)],
                                    start=not bias_pe,
                                    stop=True,
                                )
                            if bias_pe:
                                if dve_cols:
                                    nc.vector.tensor_copy(
                                        ot[:, s * FPC: s * FPC + dve_cols],
                                        ps[:, h * FPC: h * FPC + dve_cols],
                                    )
                                if dve_cols < FPC:
                                    nc.scalar.copy(
                                        ot[:, s * FPC + dve_cols:
                                           (s + 1) * FPC],
                                        ps[:, h * FPC + dve_cols:
                                           (h + 1) * FPC],
                                    )
                        if not bias_pe:
                            nc.vector.tensor_add(
                                ot[:, s0 * FPC: (s0 + pair) * FPC],
                                ps[:], bt[:],
                            )
                    nc.scalar.dma_start(out=out3[c], in_=ot[:])
    nc.compile()
    return nc


def prep_in_maps_v2(x, weight, bias, ch: int = 2048, mode: str = "dmat"):
    """Per-core fp16 input maps for build_nc_v2."""
    x2 = np.asarray(x, np.float32).reshape(TOK, IN_F)
    w = np.asarray(weight, np.float32)
    b = np.asarray(bias, np.float32)
    nchunk = TOK // ch
    sub = ch // 128
    ones = np.ones((1, 128), np.float16)
    ident = np.eye(128, dtype=np.float16)
    maps = []
    for m in range(NCORES):
        xs = x2[:, m * FPC:(m + 1) * FPC].astype(np.float16)   # [TOK, 512]
        wm = w[m * BPC:(m + 1) * BPC]                          # [16, 32, 32]
        wg = np.zeros((128, FPC), np.float16)
        for g in range(GROUPS):
            for a in range(BLOCKS_PER_GROUP):
                wg[a * 32:(a + 1) * 32,
                   g * 128 + a * 32: g * 128 + (a + 1) * 32] = wm[4 * g + a]
        b1 = b[m * BPC:(m + 1) * BPC].reshape(1, FPC).astype(np.float16)
        btm = np.ascontiguousarray(
            np.broadcast_to(b[m * BPC:(m + 1) * BPC].reshape(FPC), (128, FPC))
        ).astype(np.float32)
        mp = {"wt": wg, "b1": b1, "on": ones, "bt": btm}
        if mode == "dmat":
            # row (c, g, s*128+p) of xg = features [g*128:(g+1)*128] of
            # original token c*ch + p*sub + s
            xgm = xs.reshape(nchunk, 128, sub, GROUPS, 128)    # [c,p,s,g,u]
            xgm = np.ascontiguousarray(xgm.transpose(0, 3, 2, 1, 4))
            mp["xg"] = xgm.reshape(nchunk, GROUPS * ch, 128)
        else:
            mp["xs"] = xs
            mp["idn"] = ident
        maps.append(mp)
    return maps


def kernel_v2(inputs, ch: int = 2048, mode: str = "dmat", **bkw) -> np.ndarray:
    from concourse.bass_utils import run_bass_kernel_spmd

    nc = build_nc_v2(ch=ch, mode=mode, **bkw)
    in_maps = prep_in_maps_v2(inputs["x"], inputs["weight"], inputs["bias"],
                              ch=ch, mode=mode)
    res = run_bass_kernel_spmd(nc, in_maps, core_ids=list(range(NCORES)))
    outs = [res.results[m]["out"].astype(np.float32) for m in range(NCORES)]
    full = np.concatenate(outs, axis=1)           # [16384, 4096]
    return full.reshape(B, S, OUT_F)


def build_nc(
    tok: int = TOK,
    chunk_tok: int = 1024,
    reps: int = 1,
    loop_reps: int = 1,
    use_f32r: bool = False,
    variant: str = "full",      # full | dma | nomm | notr  (bisection variants)
    copy_engine: str = "vector",  # engine for the xT PSUM->SBUF copy
):
    """Build the per-core Bass program (SPMD: same program, per-core data).

    reps: python-unrolled repetitions of the whole pass (for timing).
    loop_reps: hardware For_i loop repetitions of the whole pass (for timing
    with constant instruction count).
    use_f32r: stream operands as float32r (same bits as fp32, faster PE
    streaming mode) and run the matmuls as zero-padded pairs with a 256-wide
    moving dim, where f32r hits 1 cycle/row instead of fp32's 4.
    """
    assert tok % chunk_tok == 0 and chunk_tok % 128 == 0
    nchunk = tok // chunk_tok
    sub = chunk_tok // 128     # 128-token sub-chunks per chunk
    XD = mybir.dt.float32r if use_f32r else F32

    nc = bacc.Bacc(
        "TRN2", target_bir_lowering=False, debug=False, num_devices=NCORES
    )
    xs = nc.dram_tensor("xs", [tok, FPC], XD, kind="ExternalInput").ap()
    if use_f32r:
        wpad = nc.dram_tensor(
            "wpad", [GROUPS, 128, 256], XD, kind="ExternalInput"
        ).ap()
    else:
        wbd = nc.dram_tensor("wbd", [GROUPS, 128, 128], F32, kind="ExternalInput").ap()
    bb = nc.dram_tensor("bb", [128, FPC], F32, kind="ExternalInput").ap()
    idn = nc.dram_tensor("idn", [128, 128], XD, kind="ExternalInput").ap()
    out = nc.dram_tensor("out", [tok, FPC], F32, kind="ExternalOutput").ap()

    xs3 = xs.rearrange("(c a p) f -> c p a f", a=sub, p=128)
    out3 = out.rearrange("(c a p) f -> c p a f", a=sub, p=128)

    with tile.TileContext(nc) as tc:
        with (
            tc.tile_pool(name="const", bufs=1) as cpool,
            tc.tile_pool(name="xin", bufs=2) as xpool,
            tc.tile_pool(name="oout", bufs=2) as opool,
            tc.tile_pool(name="xt", bufs=3) as xtpool,
            tc.tile_pool(name="ps", bufs=2, space="PSUM") as pspool,
        ):
            if use_f32r:
                wt = cpool.tile([128, GROUPS * 256], XD)
                nc.sync.dma_start(
                    out=wt[:].rearrange("p (g m) -> p g m", g=GROUPS),
                    in_=wpad.rearrange("g k m -> k g m"),
                )
            else:
                wt = cpool.tile([128, GROUPS * 128], F32)
                nc.sync.dma_start(
                    out=wt[:].rearrange("p (g m) -> p g m", g=GROUPS),
                    in_=wbd.rearrange("g k m -> k g m"),
                )
            bt = cpool.tile([128, FPC], F32)
            nc.sync.dma_start(out=bt[:], in_=bb)
            it = cpool.tile([128, 128], XD)
            nc.sync.dma_start(out=it[:], in_=idn)

            import contextlib

            loop_ctx = (
                tc.For_i(
                    0,
                    loop_reps,
                    1,
                    hint_engines=(mybir.EngineType.PE, mybir.EngineType.Activation),
                )
                if loop_reps > 1
                else contextlib.nullcontext()
            )
            with loop_ctx:
                for _ in range(reps):
                    for c in range(nchunk):
                        x_in = xpool.tile([128, sub * FPC], XD)
                        nc.sync.dma_start(
                            out=x_in[:].rearrange("p (a f) -> p a f", a=sub),
                            in_=xs3[c],
                        )
                        if variant == "dma":
                            nc.scalar.dma_start(
                                out=out3[c],
                                in_=x_in[:].rearrange("p (a f) -> p a f", a=sub),
                            )
                            continue
                        cp_fn = (
                            nc.scalar.copy
                            if copy_engine == "scalar"
                            else nc.vector.tensor_copy
                        )
                        ot = opool.tile([128, sub * FPC], F32)
                        for s in range(sub):
                            if variant != "notr":
                                xT_ps = pspool.tile([128, FPC], XD)
                                for g in range(GROUPS):
                                    nc.tensor.transpose(
                                        xT_ps[:, bass.ts(g, 128)],
                                        x_in[
                                            :,
                                            s * FPC + g * 128 : s * FPC + (g + 1) * 128,
                                        ],
                                        it[:],
                                    )
                                xT_sb = xtpool.tile([128, FPC], XD)
                                cp_fn(xT_sb[:], xT_ps[:])
                            else:
                                xT_sb = x_in[:, bass.ts(s, FPC)]
                            if variant == "nomm":
                                nc.vector.tensor_add(
                                    ot[:, bass.ts(s, FPC)], xT_ps[:], bt[:]
                                )
                                continue
                            o_ps = pspool.tile([128, FPC], F32)
                            if use_f32r:
                                for p in range(GROUPS // 2):
                                    for h in range(2):
                                        nc.tensor.matmul(
                                            o_ps[:, bass.ts(p, 256)],
                                            lhsT=xT_sb[:, bass.ts(2 * p + h, 128)],
                                            rhs=wt[:, bass.ts(2 * p + h, 256)],
                                            start=(h == 0),
                                            stop=(h == 1),
                                        )
                            else:
                                for g in range(GROUPS):
                                    nc.tensor.matmul(
                                        o_ps[:, bass.ts(g, 128)],
                                        lhsT=xT_sb[:, bass.ts(g, 128)],
                                        rhs=wt[:, bass.ts(g, 128)],
                                        start=True,
                                        stop=True,
                                    )
                            nc.vector.tensor_add(
                                ot[:, bass.ts(s, FPC)], o_ps[:], bt[:]
                            )
                        nc.scalar.dma_start(
                            out=out3[c],
                            in_=ot[:].rearrange("p (a f) -> p a f", a=sub),
                        )
    nc.compile()
    return nc


def build_nc_tok(
    tpc: int = TOK // NCORES,
    loop_reps: int = 1,
    use_f32r: bool = False,
    qf: int = 1024,             # features per PSUM quarter (multiple of 256)
    psum_bufs: int = 2,
    variant: str = "full",      # full | dma
    mm_transpose_mode: bool = False,  # run matmuls with is_transpose=True
):
    """Token-sharded per-core program: each core owns tpc tokens x all 4096
    features. DMA is fully contiguous (16 KB per partition per transfer)."""
    assert tpc % 128 == 0
    nsub = tpc // 128
    ngrp = IN_F // 128          # 32 groups of 128 features
    nq = IN_F // qf             # PSUM quarters per sub-chunk
    gq = qf // 128              # groups per quarter
    XD = mybir.dt.float32r if use_f32r else F32

    nc = bacc.Bacc(
        "TRN2", target_bir_lowering=False, debug=False, num_devices=NCORES
    )
    xs = nc.dram_tensor("xs", [tpc, IN_F], XD, kind="ExternalInput").ap()
    if use_f32r:
        wpad = nc.dram_tensor(
            "wpad", [ngrp, 128, 256], XD, kind="ExternalInput"
        ).ap()
    else:
        wbd = nc.dram_tensor("wbd", [ngrp, 128, 128], F32, kind="ExternalInput").ap()
    bb = nc.dram_tensor("bb", [128, IN_F], F32, kind="ExternalInput").ap()
    idn = nc.dram_tensor("idn", [128, 128], XD, kind="ExternalInput").ap()
    out = nc.dram_tensor("out", [tpc, IN_F], F32, kind="ExternalOutput").ap()

    xs2 = xs.rearrange("(c p) f -> c p f", p=128)
    out2 = out.rearrange("(c p) f -> c p f", p=128)

    with tile.TileContext(nc) as tc:
        with (
            tc.tile_pool(name="const", bufs=1) as cpool,
            tc.tile_pool(name="xin", bufs=3) as xpool,
            tc.tile_pool(name="oout", bufs=3) as opool,
            tc.tile_pool(name="xt", bufs=3) as xtpool,
            tc.tile_pool(name="ps", bufs=psum_bufs, space="PSUM") as pspool,
        ):
            if use_f32r:
                wt = cpool.tile([128, ngrp * 256], XD)
                nc.sync.dma_start(
                    out=wt[:].rearrange("p (g m) -> p g m", g=ngrp),
                    in_=wpad.rearrange("g k m -> k g m"),
                )
            else:
                wt = cpool.tile([128, ngrp * 128], F32)
                nc.sync.dma_start(
                    out=wt[:].rearrange("p (g m) -> p g m", g=ngrp),
                    in_=wbd.rearrange("g k m -> k g m"),
                )
            bt = cpool.tile([128, IN_F], F32)
            nc.sync.dma_start(out=bt[:], in_=bb)
            it = cpool.tile([128, 128], XD)
            nc.sync.dma_start(out=it[:], in_=idn)

            import contextlib

            loop_ctx = (
                tc.For_i(
                    0,
                    loop_reps,
                    1,
                    hint_engines=(mybir.EngineType.PE, mybir.EngineType.DVE),
                )
                if loop_reps > 1
                else contextlib.nullcontext()
            )
            with loop_ctx:
                for c in range(nsub):
                    x_in = xpool.tile([128, IN_F], XD)
                    nc.sync.dma_start(out=x_in[:], in_=xs2[c])
                    if variant == "dma":
                        nc.scalar.dma_start(out=out2[c], in_=x_in[:])
                        continue
                    ot = opool.tile([128, IN_F], F32)
                    for q in range(nq):
                        xT_ps = pspool.tile([128, qf], XD)
                        for g in range(gq):
                            nc.tensor.transpose(
                                xT_ps[:, bass.ts(g, 128)],
                                x_in[:, q * qf + g * 128 : q * qf + (g + 1) * 128],
                                it[:],
                            )
                        xT_sb = xtpool.tile([128, qf], XD)
                        nc.vector.tensor_copy(xT_sb[:], xT_ps[:])
                        o_ps = pspool.tile([128, qf], F32)
                        if use_f32r:
                            for p in range(gq // 2):
                                for h in range(2):
                                    nc.tensor.matmul(
                                        o_ps[:, bass.ts(p, 256)],
                                        lhsT=xT_sb[:, bass.ts(2 * p + h, 128)],
                                        rhs=wt[
                                            :,
                                            (q * gq + 2 * p + h)
                                            * 256 : (q * gq + 2 * p + h + 1)
                                            * 256,
                                        ],
                                        start=(h == 0),
                                        stop=(h == 1),
                                    )
                        else:
                            for g in range(gq):
                                nc.tensor.matmul(
                                    o_ps[:, bass.ts(g, 128)],
                                    lhsT=xT_sb[:, bass.ts(g, 128)],
                                    rhs=wt[:, bass.ts(q * gq + g, 128)],
                                    start=True,
                                    stop=True,
                                    is_transpose=mm_transpose_mode or None,
                                )
                        nc.vector.tensor_add(
                            ot[:, bass.ts(q, qf)], o_ps[:], bt[:, bass.ts(q, qf)]
                        )
                    nc.scalar.dma_start(out=out2[c], in_=ot[:])
    nc.compile()
    return nc


def build_nc_ht(
    tpc: int = TOK // NCORES,
    loop_reps: int = 1,
    win_tok: int = 256,         # tokens per input window (one 4MB DMA each)
    psum_bufs: int = 6,
    use_f32r: bool = False,
):
    """Host-transposed per-core program: x arrives feature-major [4096, tpc],
    so features land on partitions straight from DMA — no on-chip transpose,
    no PSUM round-trip for inputs. Token-sharded across cores."""
    assert tpc % win_tok == 0 and win_tok % 128 == 0
    nwin = tpc // win_tok
    tc_per_win = win_tok // 128
    ngrp = IN_F // 128          # 32
    XD = mybir.dt.float32r if use_f32r else F32

    nc = bacc.Bacc(
        "TRN2", target_bir_lowering=False, debug=False, num_devices=NCORES
    )
    xt = nc.dram_tensor("xt", [IN_F, tpc], XD, kind="ExternalInput").ap()
    if use_f32r:
        wpad = nc.dram_tensor(
            "wpad", [ngrp, 128, 256], XD, kind="ExternalInput"
        ).ap()
    else:
        wbd = nc.dram_tensor("wbd", [ngrp, 128, 128], F32, kind="ExternalInput").ap()
    bb = nc.dram_tensor("bb", [128, IN_F], F32, kind="ExternalInput").ap()
    out = nc.dram_tensor("out", [tpc, IN_F], F32, kind="ExternalOutput").ap()

    xt4 = xt.rearrange("(g p) t -> p g t", g=ngrp, p=128)  # [128, 32, tpc]
    out2 = out.rearrange("(c p) f -> c p f", p=128)

    with tile.TileContext(nc) as tc:
        with (
            tc.tile_pool(name="const", bufs=1) as cpool,
            tc.tile_pool(name="xin", bufs=2) as xpool,
            tc.tile_pool(name="oout", bufs=2) as opool,
            tc.tile_pool(name="ps", bufs=psum_bufs, space="PSUM") as pspool,
        ):
            if use_f32r:
                wt = cpool.tile([128, ngrp * 256], XD)
                nc.sync.dma_start(
                    out=wt[:].rearrange("p (g m) -> p g m", g=ngrp),
                    in_=wpad.rearrange("g k m -> k g m"),
                )
            else:
                wt = cpool.tile([128, ngrp * 128], F32)
                nc.sync.dma_start(
                    out=wt[:].rearrange("p (g m) -> p g m", g=ngrp),
                    in_=wbd.rearrange("g k m -> k g m"),
                )
            bt = cpool.tile([128, IN_F], F32)
            nc.sync.dma_start(out=bt[:], in_=bb)

            import contextlib

            loop_ctx = (
                tc.For_i(
                    0,
                    loop_reps,
                    1,
                    hint_engines=(mybir.EngineType.PE, mybir.EngineType.DVE),
                )
                if loop_reps > 1
                else contextlib.nullcontext()
            )
            with loop_ctx:
                for w in range(nwin):
                    xw = xpool.tile([128, ngrp * win_tok], XD)
                    nc.sync.dma_start(
                        out=xw[:].rearrange("p (g t) -> p g t", g=ngrp),
                        in_=xt4[:, :, w * win_tok : (w + 1) * win_tok],
                    )
                    for tci in range(tc_per_win):
                        ot = opool.tile([128, IN_F], F32)
                        for q in range(IN_F // 512):
                            o_ps = pspool.tile([128, 512], F32)
                            if use_f32r:
                                for p in range(2):
                                    for h in range(2):
                                        g = q * 4 + 2 * p + h
                                        nc.tensor.matmul(
                                            o_ps[:, bass.ts(p, 256)],
                                            lhsT=xw[
                                                :,
                                                g * win_tok
                                                + tci * 128 : g * win_tok
                                                + tci * 128
                                                + 128,
                                            ],
                                            rhs=wt[:, bass.ts(g, 256)],
                                            start=(h == 0),
                                            stop=(h == 1),
                                        )
                            else:
                                for j in range(4):
                                    g = q * 4 + j
                                    nc.tensor.matmul(
                                        o_ps[:, bass.ts(j, 128)],
                                        lhsT=xw[
                                            :,
                                            g * win_tok
                                            + tci * 128 : g * win_tok
                                            + tci * 128
                                            + 128,
                                        ],
                                        rhs=wt[:, bass.ts(g, 128)],
                                        start=True,
                                        stop=True,
                                    )
                            nc.vector.tensor_add(
                                ot[:, bass.ts(q, 512)],
                                o_ps[:],
                                bt[:, bass.ts(q, 512)],
                            )
                        nc.scalar.dma_start(
                            out=out2[w * tc_per_win + tci], in_=ot[:]
                        )
    nc.compile()
    return nc


def prep_in_maps_ht(x, weight, bias, use_f32r: bool = False):
    """Host-transposed inputs: per-core feature-major x slice."""
    x = np.asarray(x, dtype=np.float32).reshape(-1, IN_F)
    weight = np.asarray(weight, dtype=np.float32)
    bias = np.asarray(bias, dtype=np.float32)
    tpc = x.shape[0] // NCORES

    ngrp = IN_F // 128
    bpg = 128 // IPB
    wg = np.zeros((ngrp, 128, 128), np.float32)
    for g in range(ngrp):
        for a in range(bpg):
            wg[g, 32 * a : 32 * a + 32, 32 * a : 32 * a + 32] = weight[bpg * g + a]
    bbm = np.ascontiguousarray(np.broadcast_to(bias.reshape(IN_F), (128, IN_F)))
    maps = []
    for m in range(NCORES):
        xtm = np.ascontiguousarray(x[m * tpc : (m + 1) * tpc].T)
        mp = {"xt": xtm, "bb": bbm}
        if use_f32r:
            wp = np.zeros((ngrp, 128, 256), np.float32)
            for qq in range(ngrp):
                h = qq % 2
                wp[qq, :, 128 * h : 128 * h + 128] = wg[qq]
            mp["wpad"] = wp
        else:
            mp["wbd"] = wg
        maps.append(mp)
    return maps


def prep_in_maps_tok(x, weight, bias):
    """Token-sharded inputs: per-core contiguous token slice, shared weights."""
    x = np.ascontiguousarray(np.asarray(x, dtype=np.float32).reshape(-1, IN_F))
    weight = np.asarray(weight, dtype=np.float32)
    bias = np.asarray(bias, dtype=np.float32)
    ident = np.eye(128, dtype=np.float32)
    tpc = x.shape[0] // NCORES

    ngrp = IN_F // 128
    bpg = 128 // IPB            # blocks per 128-feature group = 4
    wg = np.zeros((ngrp, 128, 128), np.float32)
    for g in range(ngrp):
        for a in range(bpg):
            wg[g, 32 * a : 32 * a + 32, 32 * a : 32 * a + 32] = weight[bpg * g + a]
    wp = np.zeros((ngrp, 128, 256), np.float32)
    for qq in range(ngrp):
        h = qq % 2
        wp[qq, :, 128 * h : 128 * h + 128] = wg[qq]
    bbm = np.ascontiguousarray(
        np.broadcast_to(bias.reshape(IN_F), (128, IN_F))
    )
    return [
        {
            "xs": x[m * tpc : (m + 1) * tpc],
            "wbd": wg,
            "wpad": wp,
            "bb": bbm,
            "idn": ident,
        }
        for m in range(NCORES)
    ]


def prep_in_maps(x, weight, bias, tok: int = TOK):
    """Split full inputs into 8 per-core input maps (host-side numpy)."""
    x = np.asarray(x, dtype=np.float32).reshape(-1, IN_F)[:tok]
    weight = np.asarray(weight, dtype=np.float32)
    bias = np.asarray(bias, dtype=np.float32)
    ident = np.eye(128, dtype=np.float32)

    in_maps = []
    for m in range(NCORES):
        xs = np.ascontiguousarray(x[:, m * FPC : (m + 1) * FPC])
        wm = weight[m * BPC : (m + 1) * BPC]          # [16, 32, 32]
        wg = np.zeros((GROUPS, 128, 128), np.float32)
        for g in range(GROUPS):
            for a in range(BLOCKS_PER_GROUP):
                wg[g, 32 * a : 32 * a + 32, 32 * a : 32 * a + 32] = wm[
                    BLOCKS_PER_GROUP * g + a
                ]
        # zero-padded pairs for the f32r N=256 matmul path: entry q = 2p+h
        # holds group (2p+h)'s weights in column half h, zeros in the other.
        wp = np.zeros((GROUPS, 128, 256), np.float32)
        for q in range(GROUPS):
            h = q % 2
            wp[q, :, 128 * h : 128 * h + 128] = wg[q]
        bm = bias[m * BPC : (m + 1) * BPC].reshape(FPC)
        bbm = np.ascontiguousarray(np.broadcast_to(bm, (128, FPC)))
        in_maps.append({"xs": xs, "wbd": wg, "wpad": wp, "bb": bbm, "idn": ident})
    return in_maps


# Best measured config (pet = PE-transpose path, fp16 compute, DVE bias-add,
# 3x buffering, transposes emitted 2 subs ahead of matmuls so PE never
# stalls on the ACT xT copy, chunk DMAs split in 2 for faster ramp/drain):
# ~95 us/core on HW, at the 32 MB/core DMA floor (DMA-only passthrough of
# the same tiles measures ~94 us).
BEST = dict(ch=2048, mode="pet", bias_pe=False, xbufs=3, obufs=3, skew=2,
            isplit=2, osplit=2)


def kernel(**inputs) -> np.ndarray:
    from concourse.bass_utils import run_bass_kernel_spmd

    nc = build_nc_v2(**BEST)
    in_maps = prep_in_maps_v2(inputs["x"], inputs["weight"], inputs["bias"],
                              ch=BEST["ch"], mode=BEST["mode"])
    res = run_bass_kernel_spmd(nc, in_maps, core_ids=list(range(NCORES)))
    outs = [res.results[m]["out"].astype(np.float32) for m in range(NCORES)]
    full = np.concatenate(outs, axis=1)           # [16384, 4096]
    return full.reshape(B, S, OUT_F)

